# revision 4
# baseline (speedup 1.0000x reference)
"""Trainium2 Bass kernel for nn_AuxLoss (aux CE loss + erf regularizer, segment-
mean over K=10 classes), data-parallel over 8 NeuronCores.

Math (per reference):
  f(u)      = 0.5 - 0.5*erf((-0.5 - u)/(sigma*sqrt2)) = 0.5 + 0.5*erf((u+0.5)*sqrt2)
  row_reg_n = sum_d f(u[n,d])
  row_ce_n  = logsumexp(yg[n,:]) - yg[n, yhat[n]]
  per-class means over rows with yhat==k, averaged over present classes:
  out = mean_k(seg_ce/cnt) + lmbd * mean_k(seg_reg/(cnt*D))

Device strategy per core (131072 rows):
  - partition p holds a contiguous slab of 1024 rows -> fully contiguous DMAs
  - 8 chunks of 128 rows/partition; per chunk:
      ACT: erf(sqrt2*u + sqrt2/2) -> bf16   (the 0.5+0.5* affine is folded into
           the final fixup via counts: seg_f = 0.5*D*cnt + 0.5*seg_erf)
      ACT: exp(yg); DVE: sumexp; ACT: ln -> lse
      DVE: onehot[p,r,c] = (yhat==c) via iota compare (bf16)
      DVE: onehot*yg -> bf16 (diagonal trick: column-sum of its segment-matmul
           block equals seg of yg[n,yhat[n]])
      PE : per 128-row group g, onehot[:,g,:] (stationary 128x10) matmuls
           against erf[:,g,:] (64 cols) and [oh*yg | lse | ones] (12 cols),
           accumulating [10,64]+[10,12] in PSUM across all 1024 groups
  - AllReduce the 10x76 f32 accumulators across 8 cores (3 KB)
  - final per-class means + masking + lmbd combine on-device; scalar out
"""

import math
import sys

if "/opt/trn_rl_repo" not in sys.path:
    sys.path.insert(0, "/opt/trn_rl_repo")

import numpy as np

N_CORES = 8
N_FULL = 1048576
C = 10
D = 64
P = 128
ROWS_PER_CORE = N_FULL // N_CORES  # 131072
SQ2 = math.sqrt(2.0)
SM_COLS = C + 2  # [oh*yg (10) | lse | ones]
ACC_COLS = D + SM_COLS  # 76


def build(rows_per_core=ROWS_PER_CORE, w=128):
    """Build + compile the 8-core Bacc graph. w = rows per partition per chunk."""
    from concourse import bacc, mybir, tile

    f32 = mybir.dt.float32
    bf16 = mybir.dt.bfloat16
    i32 = mybir.dt.int32
    FT = mybir.ActivationFunctionType
    ALU = mybir.AluOpType
    AX = mybir.AxisListType

    rpp = rows_per_core // P  # rows per partition
    assert rpp * P == rows_per_core
    nch = rpp // w
    assert nch * w == rpp

    nc = bacc.Bacc("TRN2", target_bir_lowering=False, debug=False, num_devices=N_CORES)

    yh_d = nc.dram_tensor("yhat", [rows_per_core], i32, kind="ExternalInput")
    yg_d = nc.dram_tensor("yg", [rows_per_core, C], f32, kind="ExternalInput")
    u_d = nc.dram_tensor("u_zg", [rows_per_core, D], f32, kind="ExternalInput")
    lm_d = nc.dram_tensor("lmbd", [1, 1], f32, kind="ExternalInput")
    out_d = nc.dram_tensor("out", [1, 1], f32, kind="ExternalOutput")
    cc_in = nc.dram_tensor("cc_in", [C, ACC_COLS], f32)
    cc_out = nc.dram_tensor("cc_out", [C, ACC_COLS], f32, addr_space="Shared")

    u_v = u_d[:].rearrange("(p r) d -> p r d", p=P)  # [128, rpp, 64]
    yg_v = yg_d[:].rearrange("(p r) c -> p r c", p=P)  # [128, rpp, 10]
    yh_v = yh_d[:].rearrange("(p r) -> p r", p=P)  # [128, rpp]

    with tile.TileContext(nc) as tc:
        with (
            tc.tile_pool(name="const", bufs=1) as constp,
            tc.tile_pool(name="io", bufs=2) as iop,
            tc.tile_pool(name="work", bufs=2) as workp,
            tc.tile_pool(name="psum", bufs=1, space="PSUM") as psump,
            tc.tile_pool(name="fin", bufs=1) as finp,
        ):
            # --- constants ---
            erf_bias = constp.tile([P, 1], f32)
            nc.vector.memset(erf_bias[:], 0.5 * SQ2)
            iota_f = constp.tile([P, 1, C], f32)
            nc.gpsimd.iota(
                iota_f[:, 0, :],
                [[1, C]],
                channel_multiplier=0,
                allow_small_or_imprecise_dtypes=True,
            )

            yh_i = constp.tile([P, rpp], i32)
            nc.sync.dma_start(yh_i[:], yh_v)
            yh_f = constp.tile([P, rpp], f32)
            nc.vector.tensor_copy(yh_f[:], yh_i[:])

            ps_erf = psump.tile([C, D], f32)
            ps_sm = psump.tile([C, SM_COLS], f32)

            # --- main streaming loop ---
            for ci in range(nch):
                r0, r1 = ci * w, (ci + 1) * w

                u_t = iop.tile([P, w, D], f32, name="u_t")
                nc.sync.dma_start(u_t[:], u_v[:, r0:r1, :])
                yg_t = iop.tile([P, w, C], f32, name="yg_t")
                nc.sync.dma_start(yg_t[:], yg_v[:, r0:r1, :])

                erf_t = workp.tile([P, w, D], bf16, name="erf_t")
                nc.scalar.activation(
                    erf_t[:], u_t[:], FT.Erf, bias=erf_bias[:], scale=SQ2
                )

                oh_t = workp.tile([P, w, C], bf16, name="oh_t")
                nc.vector.tensor_tensor(
                    oh_t[:],
                    yh_f[:, r0:r1].broadcast_to([P, w, C]),
                    iota_f[:].broadcast_to([P, w, C]),
                    ALU.is_equal,
                )

                exp_t = workp.tile([P, w, C], f32, name="exp_t")
                nc.scalar.activation(exp_t[:], yg_t[:], FT.Exp)
                sume_t = workp.tile([P, w], f32, name="sume_t")
                nc.vector.reduce_sum(sume_t[:], exp_t[:], axis=AX.X)

                combo_t = workp.tile([P, w, SM_COLS], bf16, name="combo_t")
                nc.vector.tensor_tensor(
                    combo_t[:, :, 0:C], oh_t[:], yg_t[:], ALU.mult
                )
                nc.scalar.activation(combo_t[:, :, C], sume_t[:], FT.Ln)
                nc.vector.memset(combo_t[:, :, C + 1], 1.0)

                for g in range(w):
                    first = ci == 0 and g == 0
                    last = ci == nch - 1 and g == w - 1
                    nc.tensor.matmul(
                        ps_erf[:],
                        oh_t[:, g, :],
                        erf_t[:, g, :],
                        start=first,
                        stop=last,
                    )
                    nc.tensor.matmul(
                        ps_sm[:],
                        oh_t[:, g, :],
                        combo_t[:, g, :],
                        start=first,
                        stop=last,
                    )

            # --- drain accumulators, all-reduce across the 8 cores ---
            acc = finp.tile([C, ACC_COLS], f32)
            nc.vector.tensor_copy(acc[:, 0:D], ps_erf[:])
            nc.vector.tensor_copy(acc[:, D:ACC_COLS], ps_sm[:])
            nc.sync.dma_start(cc_in[:], acc[:])
            nc.gpsimd.collective_compute(
                "AllReduce",
                ALU.add,
                replica_groups=[list(range(N_CORES))],
                ins=[cc_in[:].opt()],
                outs=[cc_out[:].opt()],
            )

            # --- final per-class means on partition 0 ---
            finv = finp.tile([1, C * ACC_COLS], f32)
            nc.sync.dma_start(
                finv[:], cc_out[:].rearrange("(o k) c -> o (k c)", o=1)
            )
            f3 = finv[:].rearrange("p (k c) -> p k c", k=C)  # [1, 10, 76]

            e10 = finp.tile([1, C], f32)  # sum_d seg_erf
            nc.vector.reduce_sum(e10[:], f3[:, :, 0:D], axis=AX.X)
            p10 = finp.tile([1, C], f32)  # seg of yg[n, yhat]
            nc.vector.reduce_sum(p10[:], f3[:, :, D : D + C], axis=AX.X)
            ce10 = finp.tile([1, C], f32)  # seg_ce = seg_lse - seg_picked
            nc.vector.tensor_sub(ce10[:], f3[:, :, D + C], p10[:])
            cn10 = f3[:, :, D + C + 1]  # counts, strided [1, 10]

            mask = finp.tile([1, C], f32)
            nc.vector.tensor_scalar(mask[:], cn10, 0.0, None, ALU.is_gt)
            one_m = finp.tile([1, C], f32)
            nc.vector.tensor_scalar(one_m[:], mask[:], -1.0, 1.0, ALU.mult, ALU.add)
            den = finp.tile([1, C], f32)  # counts, with 1 where absent
            nc.vector.tensor_add(den[:], one_m[:], cn10)
            rinv = finp.tile([1, C], f32)
            nc.vector.reciprocal(rinv[:], den[:])

            # reg_c = (0.5*D*cnt + 0.5*seg_erf) / (cnt*D) = (0.5*cnt + e10/(2D)) / cnt
            hc = finp.tile([1, C], f32)
            nc.vector.tensor_scalar(hc[:], cn10, 0.5, None, ALU.mult)
            rnum = finp.tile([1, C], f32)
            nc.vector.scalar_tensor_tensor(
                rnum[:], e10[:], 1.0 / (2.0 * D), hc[:], ALU.mult, ALU.add
            )
            regc = finp.tile([1, C], f32)
            nc.vector.tensor_mul(regc[:], rnum[:], rinv[:])
            auxc = finp.tile([1, C], f32)
            nc.vector.tensor_mul(auxc[:], ce10[:], rinv[:])

            nuq = finp.tile([1, 1], f32)
            nc.vector.reduce_sum(nuq[:], mask[:], axis=AX.X)
            ninv = finp.tile([1, 1], f32)
            nc.vector.reciprocal(ninv[:], nuq[:])
            sreg = finp.tile([1, 1], f32)
            nc.vector.reduce_sum(sreg[:], regc[:], axis=AX.X)
            saux = finp.tile([1, 1], f32)
            nc.vector.reduce_sum(saux[:], auxc[:], axis=AX.X)

            lm_t = finp.tile([1, 1], f32)
            nc.sync.dma_start(lm_t[:], lm_d[:])
            t1 = finp.tile([1, 1], f32)
            nc.vector.tensor_mul(t1[:], sreg[:], lm_t[:])
            t2 = finp.tile([1, 1], f32)
            nc.vector.tensor_add(t2[:], t1[:], saux[:])
            res = finp.tile([1, 1], f32)
            nc.vector.tensor_mul(res[:], t2[:], ninv[:])
            nc.sync.dma_start(out_d[:], res[:])

    nc.compile()
    return nc


_NC_CACHE = {}


def _get_nc(rows_per_core=ROWS_PER_CORE, w=128):
    key = (rows_per_core, w)
    if key not in _NC_CACHE:
        _NC_CACHE[key] = build(rows_per_core, w)
    return _NC_CACHE[key]


def make_in_maps(yhat, yg, u_zg, lmbd, rows_per_core=ROWS_PER_CORE):
    yhat = np.ascontiguousarray(np.asarray(yhat).astype(np.int32))
    yg = np.ascontiguousarray(np.asarray(yg, dtype=np.float32))
    u_zg = np.ascontiguousarray(np.asarray(u_zg, dtype=np.float32))
    lmbd = np.asarray(lmbd, dtype=np.float32).reshape(1, 1)
    n = yhat.shape[0]
    assert n == rows_per_core * N_CORES
    in_maps = []
    for i in range(N_CORES):
        s = slice(i * rows_per_core, (i + 1) * rows_per_core)
        in_maps.append(
            {"yhat": yhat[s], "yg": yg[s], "u_zg": u_zg[s], "lmbd": lmbd}
        )
    return in_maps


def run(yhat, yg, u_zg, lmbd, trace=False, rows_per_core=ROWS_PER_CORE, w=128):
    from concourse import bass_utils

    nc = _get_nc(rows_per_core, w)
    in_maps = make_in_maps(yhat, yg, u_zg, lmbd, rows_per_core)
    res = bass_utils.run_bass_kernel_spmd(
        nc, in_maps, core_ids=list(range(N_CORES)), trace=trace
    )
    val = np.float32(np.asarray(res.results[0]["out"]).reshape(())[()])
    return val, res


def kernel(yhat, yg, u_zg, lmbd):
    val, _ = run(yhat, yg, u_zg, lmbd)
    return np.asarray(val, dtype=np.float32).reshape(())
